# revision 10
# baseline (speedup 1.0000x reference)
"""Distributed causal self-attention on 8 TRN2 NeuronCores.

Strategy (tensor parallel on heads + per-batch staged AllToAll):
  - Each core owns 2 of the 16 heads: qkv projection for its heads (full
    batch/seq), causal attention in a transposed-score layout (scores
    [key, query] so softmax denominators come free from an all-ones
    64-column block in the V stationary; no row-max subtraction needed
    at these magnitudes).
  - The two local heads' score matmuls are row-tile PACKED: each head
    contracts over its real 64 q/k features only (k_h0 on partitions
    0:64, k_h1 on 64:128), so both heads stream concurrently through
    disjoint row-groups of the PE array -- the score pass costs half
    the columns of the padded-K version.
  - Both heads' score psums land in one 2-bank PSUM pair tile, so a
    single ACTIVATE exps both heads per key tile (halves the ACT
    instruction overhead, which would otherwise become the critical
    engine).
  - V is projected directly in [token, feature] layout (x tokens as the
    stationary), eliminating the PE transposes entirely.
  - The V stationary carries an all-ones 64-column block, so the PV
    matmul broadcasts the softmax denominator across psum rows 0:64 for
    free -- the normalize is then reciprocal + elementwise multiply, no
    cross-partition broadcast.
  - Unnormalized-free y is resharded head-split -> token-split with one
    AllToAll per batch; each collective fires as soon as its batch
    finishes and overlaps the next batch's compute.
  - Per 512-token block, the NEXT block's qkv matmul groups are spread
    between this block's score/PV pairs so the PE never waits for the
    exp stream; the PE queue order is pinned with ordering-only deps.
  - Each core computes the output projection for its token chunks; the
    outproj chunks fill the tail where the core would otherwise idle on
    the final half-collectives.

All TensorEngine-facing tensors keep the contraction dim on partitions
and use bf16. PSUM accumulation stays fp32.
"""

import os
import sys

sys.path.insert(0, "/opt/trn_rl_repo")

import ml_dtypes
import numpy as np

import concourse.mybir as mybir
import concourse.tile as tile
from concourse.tile import add_dep_helper
from concourse import bacc
from concourse.bass_utils import run_bass_kernel_spmd


def _install_profile_hook():
    """The RL container's antenv stub lacks axon_hooks, so bass_utils can't
    reach the NTFF profiler. Recreate the tiny set/get module and wire it to
    trn_boot's ctypes hook against libaxon_pjrt.so."""
    import types

    if "antenv.axon_hooks" in sys.modules:
        return
    try:
        import antenv
        from trn_agent_boot.trn_boot import _ntff_profile_via_ctypes

        mod = types.ModuleType("antenv.axon_hooks")
        mod._hook = None

        def set_axon_ntff_profile_hook(h):
            mod._hook = h

        def get_axon_ntff_profile_hook():
            return mod._hook

        mod.set_axon_ntff_profile_hook = set_axon_ntff_profile_hook
        mod.get_axon_ntff_profile_hook = get_axon_ntff_profile_hook
        sys.modules["antenv.axon_hooks"] = mod
        antenv.axon_hooks = mod
        hook = _ntff_profile_via_ctypes("/opt/axon/libaxon_pjrt.so")
        if hook is not None:
            mod._hook = hook
    except Exception as e:  # profiling is best-effort; execution must work
        print(f"profile hook install failed: {e}", file=sys.stderr)


B, T, D, H, DH = 4, 2048, 1024, 16, 64
BT = B * T              # 8192 tokens
N_CORES = 8
HL = H // N_CORES       # 2 heads per core
FL = HL * DH            # 128 local features
TSLICE = BT // N_CORES  # 1024 output tokens per core
CH = TSLICE // B        # 256-token ownership chunk per (rank, batch)
CH3 = CH // 2
SCALE = DH ** -0.5
F32 = mybir.dt.float32
BF16 = mybir.dt.bfloat16

IB = 512       # query block (free dim of transposed score matmuls)
NJ = T // 128  # 16 key tiles per (b, h)
PV_LAG = 2     # PV pair for key tile jt issues at score slot jt+PV_LAG


def _build():
    nc = bacc.Bacc("TRN2", target_bir_lowering=False, debug=False,
                   num_devices=N_CORES)

    xT = nc.dram_tensor("xT", [D, BT], BF16, kind="ExternalInput")
    wqkvT = nc.dram_tensor("wqkvT", [D, 3 * FL], BF16, kind="ExternalInput")
    woutT = nc.dram_tensor("woutT", [D, D], BF16, kind="ExternalInput")
    # bf16 output halves the store traffic and the final DMA drain; the
    # host upcasts, and the rounding is well inside the error budget
    out = nc.dram_tensor("out", [TSLICE, D], BF16, kind="ExternalOutput")

    xT_r = xT[:].rearrange("(o p) t -> p o t", p=128)        # [128, 8, BT]
    wqkvT_r = wqkvT[:].rearrange("(o p) f -> p o f", p=128)  # [128, 8, 384]
    woutT_r = woutT[:].rearrange("(o p) d -> p o d", p=128)  # [128, 8, 1024]

    with tile.TileContext(nc) as tc:
        from contextlib import ExitStack

        with ExitStack() as ctx:
            const = ctx.enter_context(tc.tile_pool(name="const", bufs=1))
            wpool = ctx.enter_context(tc.tile_pool(name="wpool", bufs=1))
            xpool = ctx.enter_context(tc.tile_pool(name="xpool", bufs=3))
            x0pool = ctx.enter_context(tc.tile_pool(name="x0pool", bufs=8))
            qkvpool = ctx.enter_context(tc.tile_pool(name="qkvpool", bufs=1))
            vpool = ctx.enter_context(tc.tile_pool(name="vpool", bufs=1))
            ppool = ctx.enter_context(tc.tile_pool(name="ppool", bufs=6))
            ydpool = ctx.enter_context(tc.tile_pool(name="ydpool", bufs=4))
            opool = ctx.enter_context(tc.tile_pool(name="opool", bufs=3))
            # PSUM budget: pair scores 2x2 banks + psY 3 + psA 1 = 8
            psPair = ctx.enter_context(
                tc.tile_pool(name="psPair", bufs=2, space="PSUM"))
            psY = ctx.enter_context(
                tc.tile_pool(name="psY", bufs=3, space="PSUM"))
            psA = ctx.enter_context(
                tc.tile_pool(name="psA", bufs=1, space="PSUM"))
            dram = ctx.enter_context(
                tc.tile_pool(name="dram", bufs=1, space="DRAM"))

            # ---- startup DMA order: the very first matmul only needs
            # x dc-tile 0 + wq dc 0, so those issue first; the remaining
            # x tiles split across the gpsimd/sync queues so no engine's
            # issue backlog delays the qkv ramp
            xt0_dcs = [x0pool.tile([128, IB], BF16, tag="xt0dc",
                                   name=f"xt0dc{dc}") for dc in range(8)]
            wq_dcs = [wpool.tile([128, 3 * FL], BF16, tag=f"wq{dc}",
                                 name=f"wq{dc}") for dc in range(8)]
            nc.gpsimd.dma_start(xt0_dcs[0][:], xT_r[:, 0, 0:IB])
            nc.sync.dma_start(wq_dcs[0][:], wqkvT_r[:, 0, :])
            nc.gpsimd.dma_start(xt0_dcs[2][:], xT_r[:, 2, 0:IB])
            nc.sync.dma_start(xt0_dcs[1][:], xT_r[:, 1, 0:IB])
            nc.sync.dma_start(wq_dcs[1][:], wqkvT_r[:, 1, :])
            # tri[p, f] = 1.0 where f >= p else 0 (keep key j0+p for query
            # j0+f); needed by the first diagonal mask (~13us in)
            tri = const.tile([128, 128], BF16, tag="tri")
            nc.gpsimd.memset(tri[:], 1.0)
            nc.gpsimd.affine_select(
                out=tri[:], in_=tri[:],
                compare_op=mybir.AluOpType.is_ge,
                fill=0.0, base=0,
                pattern=[[1, 128]], channel_multiplier=-1,
            )
            nc.sync.dma_start(xt0_dcs[3][:], xT_r[:, 3, 0:IB])
            nc.gpsimd.dma_start(xt0_dcs[4][:], xT_r[:, 4, 0:IB])
            nc.sync.dma_start(wq_dcs[2][:], wqkvT_r[:, 2, :])
            nc.gpsimd.dma_start(xt0_dcs[6][:], xT_r[:, 6, 0:IB])
            nc.sync.dma_start(xt0_dcs[5][:], xT_r[:, 5, 0:IB])
            nc.sync.dma_start(wq_dcs[3][:], wqkvT_r[:, 3, :])
            nc.sync.dma_start(xt0_dcs[7][:], xT_r[:, 7, 0:IB])
            for dc in range(4, 8):
                nc.sync.dma_start(wq_dcs[dc][:], wqkvT_r[:, dc, :])
            xt_pre1 = xpool.tile([128, 8, IB], BF16, tag="xt",
                                 name="xt_pre1")
            nc.gpsimd.dma_start(xt_pre1[:], xT_r[:, :, IB:2 * IB])

            # ---- persistent q/k buffers: [128, 2, T], slot 0 = q, slot
            # 1 = k. Head 0 lives on partitions 0:64, head 1 on 64:128 of
            # both slots -- the packed score matmuls contract only their
            # own head's 64 rows, so no zero padding is needed anywhere.
            qkvTs = [qkvpool.tile([128, 2, T], BF16, tag=f"qkvT{i}",
                                  name=f"qkvT{i}") for i in range(3)]
            # vt[tok, jt, 0:64] = ones (denominator rows -> psy partitions
            # 0:64, pre-broadcast for the normalize multiply), 64:128 = v.
            # ones memsets ride the idle DVE/ACT queues at startup.
            vts_bufs = []
            for i in range(4):
                vt = vpool.tile([128, NJ, 128], BF16, tag=f"vt{i}",
                                name=f"vt{i}")
                if i < 2:
                    nc.vector.memset(vt[:, :, 0:DH], 1.0)
                else:
                    # bufs 2/3 are first used by batch 1 -- the gpsimd
                    # queue clears its startup DMA backlog well before
                    nc.gpsimd.memset(vt[:, :, 0:DH], 1.0)
                vts_bufs.append(vt)

            wout_sb = wpool.tile([128, 8, D], BF16, tag="wout")

            # ---- internal DRAM for the per-batch staged AllToAll ----
            a2a_ins = [dram.tile([N_CORES, FL, CH], BF16,
                                 tag=f"a2a_in{s}", name=f"a2a_in{s}")
                       for s in range(B)]
            a2a_outs = [dram.tile([N_CORES, FL, CH], BF16,
                                  tag=f"a2a_out{s}", name=f"a2a_out{s}")
                        for s in range(B)]
            # batch 3 splits into two half-batch collectives of 128-token
            # chunks so its first half overlaps the rest of its attention
            a2a3_ins = [dram.tile([N_CORES, FL, CH3], BF16,
                                  tag=f"a2a3_in{h}", name=f"a2a3_in{h}")
                        for h in range(2)]
            a2a3_outs = [dram.tile([N_CORES, FL, CH3], BF16,
                                   tag=f"a2a3_out{h}", name=f"a2a3_out{h}")
                         for h in range(2)]

            # ---- PE queue pinning: the Tile scheduler is priority-driven
            # and would hoist dependency-free qkv/outproj matmuls ahead of
            # the attention stream; ordering-only deps pin the PE queue to
            # the emission order, which is the schedule designed below.
            prev_pe = [None]

            def _mm(*a, **k):
                inst = nc.tensor.matmul(*a, **k)
                if prev_pe[0] is not None:
                    add_dep_helper(inst.ins, prev_pe[0], sync=False,
                                   reason="pe queue order")
                prev_pe[0] = inst.ins
                return inst

            # the normalize+staging chain of attention block X is emitted
            # early in block X+1 (or at batch end), so its psum-reading
            # DVE ops never sit ahead of mask multiplies the PE waits on
            pending_norms = []

            def _flush_norms():
                while pending_norms:
                    pending_norms.pop(0)()

            def _make_norm(b, ib, hl, psy):
                def _norm():
                    # psy rows 0:64 all hold the denominator (ones block),
                    # so the reciprocal lands pre-broadcast for the mult
                    den_rec = ydpool.tile([DH, IB], F32, tag="den_rec",
                                          name=f"den_rec_{b}_{ib}_{hl}")
                    nc.vector.reciprocal_approx_fast(den_rec[:],
                                                     psy[0:DH, :])
                    yd = ydpool.tile([DH, IB], BF16, tag="yd",
                                     name=f"yd_{b}_{ib}_{hl}")
                    nc.vector.tensor_tensor(yd[:], psy[DH:128, :],
                                            den_rec[:],
                                            mybir.AluOpType.mult)
                    r0 = hl * DH
                    # staging split across both DMA-issue queues so the
                    # collective trigger isn't serialized behind one queue
                    if b < B - 1:
                        for c in range(2):
                            eng = nc.gpsimd if c == 0 else nc.sync
                            eng.dma_start(
                                a2a_ins[b][2 * ib + c, r0:r0 + DH, :],
                                yd[:, c * CH:(c + 1) * CH],
                            )
                    else:
                        # batch 3: 128-token chunks, half buffers
                        for c in range(4):
                            eng = nc.gpsimd if c % 2 == 0 else nc.sync
                            eng.dma_start(
                                a2a3_ins[ib // 2][(ib % 2) * 4 + c,
                                                  r0:r0 + DH, :],
                                yd[:, c * CH3:(c + 1) * CH3],
                            )
                return _norm

            # ---- qkv projection for one 512-token block, as 8 thunks
            # (2 half-groups each for q and k, 4 token-tile groups for v)
            # that interleave into the previous block's attention slots
            def _qkv_chunks(b, tb, qkvT, vts):
                st = {}
                sl = tb * IB

                def x_ap(dc, t0=0, t1=IB):
                    if b == 0 and tb == 0:
                        return xt0_dcs[dc][:, t0:t1]
                    if b == 0 and tb == 1:
                        return xt_pre1[:, dc, t0:t1]
                    return st["xt"][:, dc, t0:t1]

                def proj_half(ft, lo, hi):
                    def f():
                        if ft == 0 and lo == 0:
                            if not (b == 0 and tb <= 1):
                                xt = xpool.tile([128, 8, IB], BF16,
                                                tag="xt",
                                                name=f"xt_{b}_{tb}")
                                t0 = b * T + tb * IB
                                nc.sync.dma_start(xt[:],
                                                  xT_r[:, :, t0:t0 + IB])
                                st["xt"] = xt
                            if b == 0 and tb == 1:
                                # big resident load rides behind startup
                                nc.gpsimd.dma_start(wout_sb[:], woutT_r)
                        key = "psq" if ft == 0 else "psk"
                        if lo == 0:
                            st[key] = psA.tile([128, IB], F32, tag="ps",
                                               name=f"ps_{b}_{tb}_{ft}")
                        ps = st[key]
                        for dc in range(lo, hi):
                            _mm(ps[:],
                                lhsT=wq_dcs[dc][:, ft * 128:(ft + 1) * 128],
                                rhs=x_ap(dc),
                                start=(dc == 0), stop=(dc == 7))
                        if hi == 8:
                            nc.vector.tensor_copy(
                                qkvT[:, ft, sl:sl + IB], ps[:])
                    return f

                def v_mms(q4):
                    def f():
                        # direct [token, feature] projection: x tokens as
                        # the stationary, w_v columns stream -- v arrives
                        # already transposed for the PV stationary
                        if q4 == 0:
                            st["psv"] = psA.tile([128, IB], F32, tag="ps",
                                                 name=f"psv_{b}_{tb}")
                        seg = st["psv"][:, q4 * 128:(q4 + 1) * 128]
                        for dc in range(8):
                            _mm(seg,
                                lhsT=x_ap(dc, q4 * 128, (q4 + 1) * 128),
                                rhs=wq_dcs[dc][:, 256:384],
                                start=(dc == 0), stop=(dc == 7))
                    return f

                def v_copies():
                    # all copies after all v matmul groups: the psum WAR
                    # tracking is bank-granular, so interleaving copies
                    # between the quarter groups would serialize the PE
                    # behind the DVE once per quarter
                    for q4 in range(4):
                        seg = st["psv"][:, q4 * 128:(q4 + 1) * 128]
                        jt = tb * 4 + q4
                        for hl in range(HL):
                            nc.vector.tensor_copy(
                                vts[hl][:, jt, DH:128],
                                seg[:, hl * DH:(hl + 1) * DH])

                return [proj_half(0, 0, 4), proj_half(0, 4, 8),
                        proj_half(1, 0, 4), proj_half(1, 4, 8),
                        v_mms(0), v_mms(1), v_mms(2), v_mms(3),
                        v_copies]

            # ---- output projection, as per-(tt, db) chunks ----
            op_lh = {}

            def _op_chunk(s, tt, db, buf, chs, row_base, pool):
                def f():
                    key = (s, row_base)
                    if key not in op_lh:
                        lh = opool.tile([128, 8, CH], BF16, tag="lh",
                                        name=f"lh_{s}_{row_base}")
                        lh = lh[:, :, :chs]
                        nc.sync.dma_start(
                            lh[:], buf[:].rearrange("p f t -> f p t"))
                        op_lh[key] = lh
                    lh = op_lh[key]
                    nrow = min(128, chs)
                    pst = pool.tile(
                        [128, 2, IB] if pool is psPair else [128, IB],
                        F32, tag="pair" if pool is psPair else "ps",
                        name=f"pso_{s}_{row_base}_{tt}_{db}")
                    pso = pst[:, 0, :] if pool is psPair else pst
                    for fc in range(8):
                        _mm(pso[:nrow, :],
                            lhsT=lh[:, fc, tt * nrow:(tt + 1) * nrow],
                            rhs=wout_sb[:, fc, db * IB:(db + 1) * IB],
                            start=(fc == 0), stop=(fc == 7))
                    osb = opool.tile([128, IB], BF16, tag="osb",
                                     name=f"osb_{s}_{row_base}_{tt}_{db}")
                    nc.vector.tensor_copy(osb[:nrow, :], pso[:nrow, :])
                    row0 = row_base + tt * nrow
                    nc.sync.dma_start(
                        out[row0:row0 + nrow, db * IB:(db + 1) * IB],
                        osb[:nrow, :])
                return f

            def _op_chunks(s, buf=None, chs=CH, row_base=None, pool=None):
                if buf is None:
                    buf = a2a_outs[s]
                if row_base is None:
                    row_base = s * CH
                nrow = min(128, chs)
                return [_op_chunk(s, tt, db, buf, chs, row_base, pool)
                        for tt in range(chs // nrow)
                        for db in range(D // IB)]

            # ---- causal attention for one (batch, 512-token block):
            # packed score pairs + paired exp + interleaved PV, with the
            # next block's qkv chunks spread between the slots
            def _attn_block(b, ib, qkvT, vts, chunks):
                nj = 4 * (ib + 1)
                nchunk = len(chunks)
                emitted = 0
                psys = []
                p_tiles = {}

                def pv_pair(jt):
                    c0 = max(0, (jt - ib * 4) * 128)
                    p = p_tiles.pop(jt)
                    for hl in range(HL):
                        _mm(psys[hl][:, c0:], lhsT=vts[hl][:, jt, :],
                            rhs=p[:, hl, c0:],
                            start=(jt == 0), stop=(jt == nj - 1))

                for jt in range(nj):
                    c0 = max(0, (jt - ib * 4) * 128)
                    pair = psPair.tile([128, 2, IB], F32, tag="pair",
                                       name=f"pair_{b}_{ib}_{jt}")
                    for hl in range(HL):
                        lo, hi = hl * 64, (hl + 1) * 64
                        _mm(pair[:, hl, c0:],
                            lhsT=qkvT[lo:hi, 1, jt * 128:(jt + 1) * 128],
                            rhs=qkvT[lo:hi, 0, ib * IB + c0:(ib + 1) * IB],
                            start=True, stop=True)
                    p = ppool.tile([128, 2, IB], BF16, tag="p",
                                   name=f"p_{b}_{ib}_{jt}")
                    nc.scalar.activation(
                        p[:, :, c0:], pair[:, :, c0:],
                        mybir.ActivationFunctionType.Exp, scale=SCALE)
                    if jt >= ib * 4:  # diagonal: triangular mask
                        for hl in range(HL):
                            nc.vector.tensor_tensor(
                                p[:, hl, c0:c0 + 128],
                                p[:, hl, c0:c0 + 128],
                                tri[:], mybir.AluOpType.mult)
                    p_tiles[jt] = p
                    if jt == 0:
                        for hl in range(HL):
                            psys.append(psY.tile(
                                [128, IB], F32, tag="psy",
                                name=f"psy_{b}_{ib}_{hl}"))
                    if jt == 1:
                        _flush_norms()
                    want = (jt + 1) * nchunk // nj
                    while emitted < want:
                        chunks[emitted]()
                        emitted += 1
                    if jt >= PV_LAG:
                        pv_pair(jt - PV_LAG)
                for jt in range(max(0, nj - PV_LAG), nj):
                    pv_pair(jt)
                for hl in range(HL):
                    pending_norms.append(_make_norm(b, ib, hl, psys[hl]))

            # ---- main emission ----
            op_stage_chunks = (_op_chunks(0, pool=psPair)
                               + _op_chunks(1, pool=psPair)
                               + _op_chunks(2, pool=psPair)
                               + _op_chunks(B - 1, buf=a2a3_outs[0],
                                            chs=CH3, row_base=(B - 1) * CH,
                                            pool=psPair)
                               + _op_chunks(B - 1, buf=a2a3_outs[1],
                                            chs=CH3,
                                            row_base=(B - 1) * CH + CH3,
                                            pool=psPair))

            for f in _qkv_chunks(0, 0, qkvTs[0], vts_bufs[0:2]):
                f()
            for b in range(B):
                qkvT = qkvTs[b % 3]
                vts = [vts_bufs[(2 * b + hl) % 4] for hl in range(HL)]
                for tb in range(T // IB):
                    if tb < 3:
                        nxt = _qkv_chunks(b, tb + 1, qkvT, vts)
                    elif b < B - 1:
                        nq = qkvTs[(b + 1) % 3]
                        nv = [vts_bufs[(2 * b + 2 + hl) % 4]
                              for hl in range(HL)]
                        nxt = _qkv_chunks(b + 1, 0, nq, nv)
                    else:
                        # last block: keep all outproj for the tail so the
                        # PE has maximal ungated work to chew while the
                        # final half-collective is in flight
                        nxt = []
                    _attn_block(b, tb, qkvT, vts, nxt)
                    if b == B - 1 and tb % 2 == 1:
                        _flush_norms()
                        nc.gpsimd.collective_compute(
                            "AllToAll", mybir.AluOpType.bypass,
                            replica_groups=[list(range(N_CORES))],
                            ins=[a2a3_ins[tb // 2][:]],
                            outs=[a2a3_outs[tb // 2][:]],
                        )
                if b < B - 1:
                    # stage collective fires as soon as batch b's y landed
                    _flush_norms()
                    nc.gpsimd.collective_compute(
                        "AllToAll", mybir.AluOpType.bypass,
                        replica_groups=[list(range(N_CORES))],
                        ins=[a2a_ins[b][:]], outs=[a2a_outs[b][:]],
                    )
            # outproj chunks fill the final-collective wait
            for f in op_stage_chunks:
                f()

    nc.finalize()
    return nc


_NC_CACHE = {}


def _get_nc():
    if "nc" not in _NC_CACHE:
        _NC_CACHE["nc"] = _build()
    return _NC_CACHE["nc"]


def kernel(x, w_qkv, w_out):
    x = np.asarray(x, np.float32).reshape(BT, D)
    w_qkv = np.asarray(w_qkv, np.float32)
    w_out = np.asarray(w_out, np.float32)

    xT = np.ascontiguousarray(x.T).astype(ml_dtypes.bfloat16)
    woutT = np.ascontiguousarray(w_out.T).astype(ml_dtypes.bfloat16)

    in_maps = []
    for c in range(N_CORES):
        rows = []
        for t in range(3):
            for hl in range(HL):
                h = HL * c + hl
                rows.append(w_qkv[t * H * DH + h * DH:
                                  t * H * DH + (h + 1) * DH])
        wq_c = np.concatenate(rows, axis=0)  # [384, D]
        in_maps.append({
            "xT": xT,
            "wqkvT": np.ascontiguousarray(wq_c.T).astype(ml_dtypes.bfloat16),
            "woutT": woutT,
        })

    nc = _get_nc()
    do_trace = bool(os.environ.get("ATTN_TRACE"))
    if do_trace:
        _install_profile_hook()
    res = run_bass_kernel_spmd(nc, in_maps, list(range(N_CORES)),
                               trace=do_trace)
    if res.exec_time_ns is not None:
        print(f"HW exec time: {res.exec_time_ns} ns")
        _NC_CACHE["exec_time_ns"] = res.exec_time_ns
        _NC_CACHE["trace"] = res.instructions_and_trace
    # rank r's out rows: batches 0-2 are 256-token chunks (token
    # b*T + r*256 + t); batch 3 is two 128-token chunks, one per
    # half-batch (tokens 3*T + h*1024 + r*128 + t)
    full = np.empty((BT, D), np.float32)
    for c in range(N_CORES):
        o = np.asarray(res.results[c]["out"], np.float32)
        for b in range(B - 1):
            full[b * T + c * CH:(b * T) + (c + 1) * CH] = \
                o[b * CH:(b + 1) * CH]
        b3 = (B - 1) * CH
        for h in range(2):
            dst = (B - 1) * T + h * (T // 2) + c * CH3
            full[dst:dst + CH3] = o[b3 + h * CH3:b3 + (h + 1) * CH3]
    return full.reshape(B, T, D)


# revision 12
# speedup vs baseline: 1.1894x; 1.1894x over previous
"""Distributed causal self-attention on 8 TRN2 NeuronCores.

Strategy (tensor parallel on heads + per-batch staged AllToAll):
  - Each core owns 2 of the 16 heads: qkv projection for its heads (full
    batch/seq), causal attention in a transposed-score layout (scores
    [key, query] so softmax denominators come free from an all-ones
    64-column block in the V stationary; no row-max subtraction needed
    at these magnitudes).
  - The two local heads' score matmuls are row-tile PACKED: each head
    contracts over its real 64 q/k features only (k_h0 on partitions
    0:64, k_h1 on 64:128), so both heads stream concurrently through
    disjoint row-groups of the PE array -- the score pass costs half
    the columns of the padded-K version.
  - Both heads' score psums land in one 2-bank PSUM pair tile, so a
    single ACTIVATE exps both heads per key tile (halves the ACT
    instruction overhead, which would otherwise become the critical
    engine).
  - V is projected directly in [token, feature] layout (x tokens as the
    stationary), eliminating the PE transposes entirely.
  - The V stationary carries an all-ones 64-column block, so the PV
    matmul broadcasts the softmax denominator across psum rows 0:64 for
    free -- the normalize is then reciprocal + elementwise multiply, no
    cross-partition broadcast.
  - Unnormalized-free y is resharded head-split -> token-split with one
    AllToAll per batch; each collective fires as soon as its batch
    finishes and overlaps the next batch's compute.
  - Per 512-token block, the NEXT block's qkv matmul groups are spread
    between this block's score/PV pairs so the PE never waits for the
    exp stream; the PE queue order is pinned with ordering-only deps.
  - Each core computes the output projection for its token chunks; the
    outproj chunks fill the tail where the core would otherwise idle on
    the final half-collectives.

All TensorEngine-facing tensors keep the contraction dim on partitions
and use bf16. PSUM accumulation stays fp32.
"""

import os
import sys

sys.path.insert(0, "/opt/trn_rl_repo")

import ml_dtypes
import numpy as np

import concourse.mybir as mybir
import concourse.tile as tile
from concourse.tile import add_dep_helper
from concourse import bacc
from concourse.bass_utils import run_bass_kernel_spmd


def _install_profile_hook():
    """The RL container's antenv stub lacks axon_hooks, so bass_utils can't
    reach the NTFF profiler. Recreate the tiny set/get module and wire it to
    trn_boot's ctypes hook against libaxon_pjrt.so."""
    import types

    if "antenv.axon_hooks" in sys.modules:
        return
    try:
        import antenv
        from trn_agent_boot.trn_boot import _ntff_profile_via_ctypes

        mod = types.ModuleType("antenv.axon_hooks")
        mod._hook = None

        def set_axon_ntff_profile_hook(h):
            mod._hook = h

        def get_axon_ntff_profile_hook():
            return mod._hook

        mod.set_axon_ntff_profile_hook = set_axon_ntff_profile_hook
        mod.get_axon_ntff_profile_hook = get_axon_ntff_profile_hook
        sys.modules["antenv.axon_hooks"] = mod
        antenv.axon_hooks = mod
        hook = _ntff_profile_via_ctypes("/opt/axon/libaxon_pjrt.so")
        if hook is not None:
            mod._hook = hook
    except Exception as e:  # profiling is best-effort; execution must work
        print(f"profile hook install failed: {e}", file=sys.stderr)


B, T, D, H, DH = 4, 2048, 1024, 16, 64
BT = B * T              # 8192 tokens
N_CORES = 8
HL = H // N_CORES       # 2 heads per core
FL = HL * DH            # 128 local features
TSLICE = BT // N_CORES  # 1024 output tokens per core
CH = TSLICE // B        # 256-token ownership chunk per (rank, batch)
CH3 = CH // 2
SCALE = DH ** -0.5
F32 = mybir.dt.float32
BF16 = mybir.dt.bfloat16

IB = 512       # query block (free dim of transposed score matmuls)
NJ = T // 128  # 16 key tiles per (b, h)
PV_LAG = 2     # PV pair for key tile jt issues at score slot jt+PV_LAG


def _build():
    nc = bacc.Bacc("TRN2", target_bir_lowering=False, debug=False,
                   num_devices=N_CORES)

    xT = nc.dram_tensor("xT", [D, BT], BF16, kind="ExternalInput")
    wqkvT = nc.dram_tensor("wqkvT", [D, 3 * FL], BF16, kind="ExternalInput")
    woutT = nc.dram_tensor("woutT", [D, D], BF16, kind="ExternalInput")
    # bf16 output halves the store traffic and the final DMA drain; the
    # host upcasts, and the rounding is well inside the error budget
    out = nc.dram_tensor("out", [TSLICE, D], BF16, kind="ExternalOutput")

    xT_r = xT[:].rearrange("(o p) t -> p o t", p=128)        # [128, 8, BT]
    wqkvT_r = wqkvT[:].rearrange("(o p) f -> p o f", p=128)  # [128, 8, 384]
    woutT_r = woutT[:].rearrange("(o p) d -> p o d", p=128)  # [128, 8, 1024]

    with tile.TileContext(nc) as tc:
        from contextlib import ExitStack

        with ExitStack() as ctx:
            const = ctx.enter_context(tc.tile_pool(name="const", bufs=1))
            wpool = ctx.enter_context(tc.tile_pool(name="wpool", bufs=1))
            xpool = ctx.enter_context(tc.tile_pool(name="xpool", bufs=3))
            x0pool = ctx.enter_context(tc.tile_pool(name="x0pool", bufs=8))
            qkvpool = ctx.enter_context(tc.tile_pool(name="qkvpool", bufs=1))
            vpool = ctx.enter_context(tc.tile_pool(name="vpool", bufs=1))
            ppool = ctx.enter_context(tc.tile_pool(name="ppool", bufs=6))
            ydpool = ctx.enter_context(tc.tile_pool(name="ydpool", bufs=4))
            opool = ctx.enter_context(tc.tile_pool(name="opool", bufs=3))
            # PSUM budget: pair scores 2x2 banks + psY 2 + psA 2 = 8.
            # psA=2 gives the q/k/v psum rotation a full block of slack
            # before the WAR on the v copies bites.
            psPair = ctx.enter_context(
                tc.tile_pool(name="psPair", bufs=2, space="PSUM"))
            psY = ctx.enter_context(
                tc.tile_pool(name="psY", bufs=2, space="PSUM"))
            psA = ctx.enter_context(
                tc.tile_pool(name="psA", bufs=2, space="PSUM"))
            dram = ctx.enter_context(
                tc.tile_pool(name="dram", bufs=1, space="DRAM"))

            # ---- startup DMA order: the very first matmul only needs
            # x dc-tile 0 + wq dc 0, so those issue first; the remaining
            # x tiles split across the gpsimd/sync queues so no engine's
            # issue backlog delays the qkv ramp
            xt0_dcs = [x0pool.tile([128, IB], BF16, tag="xt0dc",
                                   name=f"xt0dc{dc}") for dc in range(8)]
            wq_dcs = [wpool.tile([128, 3 * FL], BF16, tag=f"wq{dc}",
                                 name=f"wq{dc}") for dc in range(8)]
            nc.gpsimd.dma_start(xt0_dcs[0][:], xT_r[:, 0, 0:IB])
            nc.sync.dma_start(wq_dcs[0][:], wqkvT_r[:, 0, :])
            nc.gpsimd.dma_start(xt0_dcs[2][:], xT_r[:, 2, 0:IB])
            nc.sync.dma_start(xt0_dcs[1][:], xT_r[:, 1, 0:IB])
            nc.sync.dma_start(wq_dcs[1][:], wqkvT_r[:, 1, :])
            # tri[p, f] = 1.0 where f >= p else 0 (keep key j0+p for query
            # j0+f); needed by the first diagonal mask (~13us in)
            tri = const.tile([128, 128], BF16, tag="tri")
            nc.gpsimd.memset(tri[:], 1.0)
            nc.gpsimd.affine_select(
                out=tri[:], in_=tri[:],
                compare_op=mybir.AluOpType.is_ge,
                fill=0.0, base=0,
                pattern=[[1, 128]], channel_multiplier=-1,
            )
            nc.sync.dma_start(xt0_dcs[3][:], xT_r[:, 3, 0:IB])
            nc.gpsimd.dma_start(xt0_dcs[4][:], xT_r[:, 4, 0:IB])
            nc.sync.dma_start(wq_dcs[2][:], wqkvT_r[:, 2, :])
            nc.gpsimd.dma_start(xt0_dcs[6][:], xT_r[:, 6, 0:IB])
            nc.sync.dma_start(xt0_dcs[5][:], xT_r[:, 5, 0:IB])
            nc.sync.dma_start(wq_dcs[3][:], wqkvT_r[:, 3, :])
            nc.sync.dma_start(xt0_dcs[7][:], xT_r[:, 7, 0:IB])
            for dc in range(4, 8):
                nc.sync.dma_start(wq_dcs[dc][:], wqkvT_r[:, dc, :])
            xt_pre1 = xpool.tile([128, 8, IB], BF16, tag="xt",
                                 name="xt_pre1")
            nc.gpsimd.dma_start(xt_pre1[:], xT_r[:, :, IB:2 * IB])

            # ---- persistent q/k buffers: [128, 2, T], slot 0 = q, slot
            # 1 = k. Head 0 lives on partitions 0:64, head 1 on 64:128 of
            # both slots -- the packed score matmuls contract only their
            # own head's 64 rows, so no zero padding is needed anywhere.
            qkvTs = [qkvpool.tile([128, 2, T], BF16, tag=f"qkvT{i}",
                                  name=f"qkvT{i}") for i in range(3)]
            # vt[tok, jt, 0:64] = ones (denominator rows -> psy partitions
            # 0:64, pre-broadcast for the normalize multiply), 64:128 = v.
            # ones memsets ride the idle DVE/ACT queues at startup.
            vts_bufs = []
            for i in range(4):
                vt = vpool.tile([128, NJ, 128], BF16, tag=f"vt{i}",
                                name=f"vt{i}")
                if i < 2:
                    nc.vector.memset(vt[:, :, 0:DH], 1.0)
                else:
                    # bufs 2/3 are first used by batch 1 -- the gpsimd
                    # queue clears its startup DMA backlog well before
                    nc.gpsimd.memset(vt[:, :, 0:DH], 1.0)
                vts_bufs.append(vt)

            wout_sb = wpool.tile([128, 8, D], BF16, tag="wout")

            # ---- internal DRAM for the per-batch staged AllToAll ----
            a2a_ins = [dram.tile([N_CORES, FL, CH], BF16,
                                 tag=f"a2a_in{s}", name=f"a2a_in{s}")
                       for s in range(B)]
            a2a_outs = [dram.tile([N_CORES, FL, CH], BF16,
                                  tag=f"a2a_out{s}", name=f"a2a_out{s}")
                        for s in range(B)]
            # batch 3 splits into two half-batch collectives of 128-token
            # chunks so its first half overlaps the rest of its attention
            a2a3_ins = [dram.tile([N_CORES, FL, CH3], BF16,
                                  tag=f"a2a3_in{h}", name=f"a2a3_in{h}")
                        for h in range(2)]
            a2a3_outs = [dram.tile([N_CORES, FL, CH3], BF16,
                                   tag=f"a2a3_out{h}", name=f"a2a3_out{h}")
                         for h in range(2)]

            # ---- PE queue pinning: the Tile scheduler is priority-driven
            # and would hoist dependency-free qkv/outproj matmuls ahead of
            # the attention stream; ordering-only deps pin the PE queue to
            # the emission order, which is the schedule designed below.
            prev_pe = [None]

            def _mm(*a, **k):
                inst = nc.tensor.matmul(*a, **k)
                if prev_pe[0] is not None:
                    add_dep_helper(inst.ins, prev_pe[0], sync=False,
                                   reason="pe queue order")
                prev_pe[0] = inst.ins
                return inst

            # the normalize+staging chain of attention block X is emitted
            # early in block X+1 (or at batch end), so its psum-reading
            # DVE ops never sit ahead of mask multiplies the PE waits on
            pending_norms = []

            def _flush_norms():
                while pending_norms:
                    pending_norms.pop(0)()

            def _make_norm(b, ib, hl, psy):
                def _norm():
                    # psy rows 0:64 all hold the denominator (ones block),
                    # so the reciprocal lands pre-broadcast for the mult
                    den_rec = ydpool.tile([DH, IB], F32, tag="den_rec",
                                          name=f"den_rec_{b}_{ib}_{hl}")
                    nc.vector.reciprocal_approx_fast(den_rec[:],
                                                     psy[0:DH, :])
                    yd = ydpool.tile([DH, IB], BF16, tag="yd",
                                     name=f"yd_{b}_{ib}_{hl}")
                    nc.vector.tensor_tensor(yd[:], psy[DH:128, :],
                                            den_rec[:],
                                            mybir.AluOpType.mult)
                    r0 = hl * DH
                    # staging stays on gpsimd: a dependency-waiting DMA
                    # issue head-of-line-blocks its queue, and sync
                    # carries the latency-critical x prefetches
                    if b < B - 1:
                        for c in range(2):
                            nc.gpsimd.dma_start(
                                a2a_ins[b][2 * ib + c, r0:r0 + DH, :],
                                yd[:, c * CH:(c + 1) * CH],
                            )
                    else:
                        # batch 3: 128-token chunks, half buffers
                        for c in range(4):
                            nc.gpsimd.dma_start(
                                a2a3_ins[ib // 2][(ib % 2) * 4 + c,
                                                  r0:r0 + DH, :],
                                yd[:, c * CH3:(c + 1) * CH3],
                            )
                return _norm

            # ---- qkv projection for one 512-token block, as 8 thunks
            # (2 half-groups each for q and k, 4 token-tile groups for v)
            # that interleave into the previous block's attention slots
            def _qkv_chunks(b, tb, qkvT, vts):
                st = {}
                sl = tb * IB

                def x_ap(dc, t0=0, t1=IB):
                    if b == 0 and tb == 0:
                        return xt0_dcs[dc][:, t0:t1]
                    if b == 0 and tb == 1:
                        return xt_pre1[:, dc, t0:t1]
                    return st["xt"][:, dc, t0:t1]

                def proj_half(ft, lo, hi):
                    def f():
                        if ft == 0 and lo == 0:
                            if not (b == 0 and tb <= 1):
                                xt = xpool.tile([128, 8, IB], BF16,
                                                tag="xt",
                                                name=f"xt_{b}_{tb}")
                                t0 = b * T + tb * IB
                                nc.sync.dma_start(xt[:],
                                                  xT_r[:, :, t0:t0 + IB])
                                st["xt"] = xt
                            if b == 0 and tb == 1:
                                # big resident load rides behind startup
                                nc.gpsimd.dma_start(wout_sb[:], woutT_r)
                        key = "psq" if ft == 0 else "psk"
                        if lo == 0:
                            st[key] = psA.tile([128, IB], F32, tag="ps",
                                               name=f"ps_{b}_{tb}_{ft}")
                        ps = st[key]
                        for dc in range(lo, hi):
                            _mm(ps[:],
                                lhsT=wq_dcs[dc][:, ft * 128:(ft + 1) * 128],
                                rhs=x_ap(dc),
                                start=(dc == 0), stop=(dc == 7))
                        if hi == 8:
                            nc.vector.tensor_copy(
                                qkvT[:, ft, sl:sl + IB], ps[:])
                    return f

                def v_mms(q4):
                    def f():
                        # direct [token, feature] projection: x tokens as
                        # the stationary, w_v columns stream -- v arrives
                        # already transposed for the PV stationary
                        if q4 == 0:
                            st["psv"] = psA.tile([128, IB], F32, tag="ps",
                                                 name=f"psv_{b}_{tb}")
                        seg = st["psv"][:, q4 * 128:(q4 + 1) * 128]
                        for dc in range(8):
                            _mm(seg,
                                lhsT=x_ap(dc, q4 * 128, (q4 + 1) * 128),
                                rhs=wq_dcs[dc][:, 256:384],
                                start=(dc == 0), stop=(dc == 7))
                    return f

                def v_copies():
                    # all copies after all v matmul groups: the psum WAR
                    # tracking is bank-granular, so interleaving copies
                    # between the quarter groups would serialize the PE
                    # behind the DVE once per quarter
                    for q4 in range(4):
                        seg = st["psv"][:, q4 * 128:(q4 + 1) * 128]
                        jt = tb * 4 + q4
                        for hl in range(HL):
                            nc.vector.tensor_copy(
                                vts[hl][:, jt, DH:128],
                                seg[:, hl * DH:(hl + 1) * DH])

                return [proj_half(0, 0, 4), proj_half(0, 4, 8),
                        proj_half(1, 0, 4), proj_half(1, 4, 8),
                        v_mms(0), v_mms(1), v_mms(2), v_mms(3),
                        v_copies]

            # ---- output projection, as per-(tt, db) chunks ----
            op_lh = {}

            def _op_chunk(s, tt, db, buf, chs, row_base, pool):
                def f():
                    key = (s, row_base)
                    if key not in op_lh:
                        lh = opool.tile([128, 8, CH], BF16, tag="lh",
                                        name=f"lh_{s}_{row_base}")
                        lh = lh[:, :, :chs]
                        nc.sync.dma_start(
                            lh[:], buf[:].rearrange("p f t -> f p t"))
                        op_lh[key] = lh
                    lh = op_lh[key]
                    nrow = min(128, chs)
                    pst = pool.tile(
                        [128, 2, IB] if pool is psPair else [128, IB],
                        F32, tag="pair" if pool is psPair else "ps",
                        name=f"pso_{s}_{row_base}_{tt}_{db}")
                    pso = pst[:, 0, :] if pool is psPair else pst
                    for fc in range(8):
                        _mm(pso[:nrow, :],
                            lhsT=lh[:, fc, tt * nrow:(tt + 1) * nrow],
                            rhs=wout_sb[:, fc, db * IB:(db + 1) * IB],
                            start=(fc == 0), stop=(fc == 7))
                    osb = opool.tile([128, IB], BF16, tag="osb",
                                     name=f"osb_{s}_{row_base}_{tt}_{db}")
                    nc.vector.tensor_copy(osb[:nrow, :], pso[:nrow, :])
                    row0 = row_base + tt * nrow
                    nc.sync.dma_start(
                        out[row0:row0 + nrow, db * IB:(db + 1) * IB],
                        osb[:nrow, :])
                return f

            def _op_chunks(s, buf=None, chs=CH, row_base=None, pool=None):
                if buf is None:
                    buf = a2a_outs[s]
                if row_base is None:
                    row_base = s * CH
                nrow = min(128, chs)
                return [_op_chunk(s, tt, db, buf, chs, row_base, pool)
                        for tt in range(chs // nrow)
                        for db in range(D // IB)]

            # ---- causal attention for one (batch, 512-token block):
            # packed score pairs + paired exp + interleaved PV, with the
            # next block's qkv chunks spread between the slots
            def _attn_block(b, ib, qkvT, vts, chunks):
                nj = 4 * (ib + 1)
                nchunk = len(chunks)
                emitted = 0
                psys = []
                p_tiles = {}

                def pv_pair(jt):
                    c0 = max(0, (jt - ib * 4) * 128)
                    p = p_tiles.pop(jt)
                    for hl in range(HL):
                        _mm(psys[hl][:, c0:], lhsT=vts[hl][:, jt, :],
                            rhs=p[:, hl, c0:],
                            start=(jt == 0), stop=(jt == nj - 1))

                for jt in range(nj):
                    c0 = max(0, (jt - ib * 4) * 128)
                    pair = psPair.tile([128, 2, IB], F32, tag="pair",
                                       name=f"pair_{b}_{ib}_{jt}")
                    for hl in range(HL):
                        lo, hi = hl * 64, (hl + 1) * 64
                        _mm(pair[:, hl, c0:],
                            lhsT=qkvT[lo:hi, 1, jt * 128:(jt + 1) * 128],
                            rhs=qkvT[lo:hi, 0, ib * IB + c0:(ib + 1) * IB],
                            start=True, stop=True)
                    p = ppool.tile([128, 2, IB], BF16, tag="p",
                                   name=f"p_{b}_{ib}_{jt}")
                    nc.scalar.activation(
                        p[:, :, c0:], pair[:, :, c0:],
                        mybir.ActivationFunctionType.Exp, scale=SCALE)
                    if jt >= ib * 4:  # diagonal: triangular mask
                        for hl in range(HL):
                            nc.vector.tensor_tensor(
                                p[:, hl, c0:c0 + 128],
                                p[:, hl, c0:c0 + 128],
                                tri[:], mybir.AluOpType.mult)
                    p_tiles[jt] = p
                    if jt == 0:
                        for hl in range(HL):
                            psys.append(psY.tile(
                                [128, IB], F32, tag="psy",
                                name=f"psy_{b}_{ib}_{hl}"))
                    if jt == 1:
                        _flush_norms()
                    want = (jt + 1) * nchunk // nj
                    while emitted < want:
                        chunks[emitted]()
                        emitted += 1
                    if jt >= PV_LAG:
                        pv_pair(jt - PV_LAG)
                for jt in range(max(0, nj - PV_LAG), nj):
                    pv_pair(jt)
                for hl in range(HL):
                    pending_norms.append(_make_norm(b, ib, hl, psys[hl]))

            # ---- main emission ----
            op_stage_chunks = (_op_chunks(0, pool=psPair)
                               + _op_chunks(1, pool=psPair)
                               + _op_chunks(2, pool=psPair)
                               + _op_chunks(B - 1, buf=a2a3_outs[0],
                                            chs=CH3, row_base=(B - 1) * CH,
                                            pool=psPair)
                               + _op_chunks(B - 1, buf=a2a3_outs[1],
                                            chs=CH3,
                                            row_base=(B - 1) * CH + CH3,
                                            pool=psPair))

            for f in _qkv_chunks(0, 0, qkvTs[0], vts_bufs[0:2]):
                f()
            for b in range(B):
                qkvT = qkvTs[b % 3]
                vts = [vts_bufs[(2 * b + hl) % 4] for hl in range(HL)]
                for tb in range(T // IB):
                    if tb < 3:
                        nxt = _qkv_chunks(b, tb + 1, qkvT, vts)
                    elif b < B - 1:
                        nq = qkvTs[(b + 1) % 3]
                        nv = [vts_bufs[(2 * b + 2 + hl) % 4]
                              for hl in range(HL)]
                        nxt = _qkv_chunks(b + 1, 0, nq, nv)
                    else:
                        # last block: keep all outproj for the tail so the
                        # PE has maximal ungated work to chew while the
                        # final half-collective is in flight
                        nxt = []
                    _attn_block(b, tb, qkvT, vts, nxt)
                    if b == B - 1 and tb % 2 == 1:
                        _flush_norms()
                        nc.gpsimd.collective_compute(
                            "AllToAll", mybir.AluOpType.bypass,
                            replica_groups=[list(range(N_CORES))],
                            ins=[a2a3_ins[tb // 2][:]],
                            outs=[a2a3_outs[tb // 2][:]],
                        )
                if b < B - 1:
                    # stage collective fires as soon as batch b's y landed
                    _flush_norms()
                    nc.gpsimd.collective_compute(
                        "AllToAll", mybir.AluOpType.bypass,
                        replica_groups=[list(range(N_CORES))],
                        ins=[a2a_ins[b][:]], outs=[a2a_outs[b][:]],
                    )
            # outproj chunks fill the final-collective wait
            for f in op_stage_chunks:
                f()

    nc.finalize()
    return nc


_NC_CACHE = {}


def _get_nc():
    if "nc" not in _NC_CACHE:
        _NC_CACHE["nc"] = _build()
    return _NC_CACHE["nc"]


def kernel(x, w_qkv, w_out):
    x = np.asarray(x, np.float32).reshape(BT, D)
    w_qkv = np.asarray(w_qkv, np.float32)
    w_out = np.asarray(w_out, np.float32)

    xT = np.ascontiguousarray(x.T).astype(ml_dtypes.bfloat16)
    woutT = np.ascontiguousarray(w_out.T).astype(ml_dtypes.bfloat16)

    in_maps = []
    for c in range(N_CORES):
        rows = []
        for t in range(3):
            for hl in range(HL):
                h = HL * c + hl
                rows.append(w_qkv[t * H * DH + h * DH:
                                  t * H * DH + (h + 1) * DH])
        wq_c = np.concatenate(rows, axis=0)  # [384, D]
        in_maps.append({
            "xT": xT,
            "wqkvT": np.ascontiguousarray(wq_c.T).astype(ml_dtypes.bfloat16),
            "woutT": woutT,
        })

    nc = _get_nc()
    do_trace = bool(os.environ.get("ATTN_TRACE"))
    if do_trace:
        _install_profile_hook()
    res = run_bass_kernel_spmd(nc, in_maps, list(range(N_CORES)),
                               trace=do_trace)
    if res.exec_time_ns is not None:
        print(f"HW exec time: {res.exec_time_ns} ns")
        _NC_CACHE["exec_time_ns"] = res.exec_time_ns
        _NC_CACHE["trace"] = res.instructions_and_trace
    # rank r's out rows: batches 0-2 are 256-token chunks (token
    # b*T + r*256 + t); batch 3 is two 128-token chunks, one per
    # half-batch (tokens 3*T + h*1024 + r*128 + t)
    full = np.empty((BT, D), np.float32)
    for c in range(N_CORES):
        o = np.asarray(res.results[c]["out"], np.float32)
        for b in range(B - 1):
            full[b * T + c * CH:(b * T) + (c + 1) * CH] = \
                o[b * CH:(b + 1) * CH]
        b3 = (B - 1) * CH
        for h in range(2):
            dst = (B - 1) * T + h * (T // 2) + c * CH3
            full[dst:dst + CH3] = o[b3 + h * CH3:b3 + (h + 1) * CH3]
    return full.reshape(B, T, D)


# revision 14
# speedup vs baseline: 1.2535x; 1.0539x over previous
"""Distributed causal self-attention on 8 TRN2 NeuronCores.

Strategy (tensor parallel on heads + per-batch staged AllToAll):
  - Each core owns 2 of the 16 heads: qkv projection for its heads (full
    batch/seq), causal attention in a transposed-score layout (scores
    [key, query] so softmax denominators come free from an all-ones
    64-column block in the V stationary; no row-max subtraction needed
    at these magnitudes).
  - The two local heads' score matmuls are row-tile PACKED: each head
    contracts over its real 64 q/k features only (k_h0 on partitions
    0:64, k_h1 on 64:128), so both heads stream concurrently through
    disjoint row-groups of the PE array -- the score pass costs half
    the columns of the padded-K version.
  - Both heads' score psums land in one 2-bank PSUM pair tile, so a
    single ACTIVATE exps both heads per key tile (halves the ACT
    instruction overhead, which would otherwise become the critical
    engine).
  - V is projected directly in [token, feature] layout (x tokens as the
    stationary), eliminating the PE transposes entirely.
  - The V stationary carries an all-ones 64-column block, so the PV
    matmul broadcasts the softmax denominator across psum rows 0:64 for
    free -- the normalize is then reciprocal + elementwise multiply, no
    cross-partition broadcast.
  - Unnormalized-free y is resharded head-split -> token-split with one
    AllToAll per batch; each collective fires as soon as its batch
    finishes and overlaps the next batch's compute.
  - Per 512-token block, the NEXT block's qkv matmul groups are spread
    between this block's score/PV pairs so the PE never waits for the
    exp stream; the PE queue order is pinned with ordering-only deps.
  - Each core computes the output projection for its token chunks; the
    outproj chunks fill the tail where the core would otherwise idle on
    the final half-collectives.

All TensorEngine-facing tensors keep the contraction dim on partitions
and use bf16. PSUM accumulation stays fp32.
"""

import os
import sys

sys.path.insert(0, "/opt/trn_rl_repo")

import ml_dtypes
import numpy as np

import concourse.mybir as mybir
import concourse.tile as tile
from concourse.tile import add_dep_helper
from concourse import bacc
from concourse.bass_utils import run_bass_kernel_spmd


def _install_profile_hook():
    """The RL container's antenv stub lacks axon_hooks, so bass_utils can't
    reach the NTFF profiler. Recreate the tiny set/get module and wire it to
    trn_boot's ctypes hook against libaxon_pjrt.so."""
    import types

    if "antenv.axon_hooks" in sys.modules:
        return
    try:
        import antenv
        from trn_agent_boot.trn_boot import _ntff_profile_via_ctypes

        mod = types.ModuleType("antenv.axon_hooks")
        mod._hook = None

        def set_axon_ntff_profile_hook(h):
            mod._hook = h

        def get_axon_ntff_profile_hook():
            return mod._hook

        mod.set_axon_ntff_profile_hook = set_axon_ntff_profile_hook
        mod.get_axon_ntff_profile_hook = get_axon_ntff_profile_hook
        sys.modules["antenv.axon_hooks"] = mod
        antenv.axon_hooks = mod
        hook = _ntff_profile_via_ctypes("/opt/axon/libaxon_pjrt.so")
        if hook is not None:
            mod._hook = hook
    except Exception as e:  # profiling is best-effort; execution must work
        print(f"profile hook install failed: {e}", file=sys.stderr)


B, T, D, H, DH = 4, 2048, 1024, 16, 64
BT = B * T              # 8192 tokens
N_CORES = 8
HL = H // N_CORES       # 2 heads per core
FL = HL * DH            # 128 local features
TSLICE = BT // N_CORES  # 1024 output tokens per core
CH = TSLICE // B        # 256-token ownership chunk per (rank, batch)
CH3 = CH // 2
SCALE = DH ** -0.5
F32 = mybir.dt.float32
BF16 = mybir.dt.bfloat16

IB = 512       # query block (free dim of transposed score matmuls)
NJ = T // 128  # 16 key tiles per (b, h)
PV_LAG = 2     # PV pair for key tile jt issues at score slot jt+PV_LAG


def _build():
    nc = bacc.Bacc("TRN2", target_bir_lowering=False, debug=False,
                   num_devices=N_CORES)

    xT = nc.dram_tensor("xT", [D, BT], BF16, kind="ExternalInput")
    wqkvT = nc.dram_tensor("wqkvT", [D, 3 * FL], BF16, kind="ExternalInput")
    woutT = nc.dram_tensor("woutT", [D, D], BF16, kind="ExternalInput")
    # bf16 output halves the store traffic and the final DMA drain; the
    # host upcasts, and the rounding is well inside the error budget
    out = nc.dram_tensor("out", [TSLICE, D], BF16, kind="ExternalOutput")

    xT_r = xT[:].rearrange("(o p) t -> p o t", p=128)        # [128, 8, BT]
    wqkvT_r = wqkvT[:].rearrange("(o p) f -> p o f", p=128)  # [128, 8, 384]
    woutT_r = woutT[:].rearrange("(o p) d -> p o d", p=128)  # [128, 8, 1024]

    with tile.TileContext(nc) as tc:
        from contextlib import ExitStack

        with ExitStack() as ctx:
            const = ctx.enter_context(tc.tile_pool(name="const", bufs=1))
            wpool = ctx.enter_context(tc.tile_pool(name="wpool", bufs=1))
            xpool = ctx.enter_context(tc.tile_pool(name="xpool", bufs=3))
            x0pool = ctx.enter_context(tc.tile_pool(name="x0pool", bufs=8))
            qkvpool = ctx.enter_context(tc.tile_pool(name="qkvpool", bufs=1))
            vpool = ctx.enter_context(tc.tile_pool(name="vpool", bufs=1))
            ppool = ctx.enter_context(tc.tile_pool(name="ppool", bufs=6))
            ydpool = ctx.enter_context(tc.tile_pool(name="ydpool", bufs=4))
            opool = ctx.enter_context(tc.tile_pool(name="opool", bufs=3))
            # PSUM budget: pair scores 2x2 banks + psY 2 + psA 2 = 8.
            # psA=2 gives the q/k/v psum rotation a full block of slack
            # before the WAR on the v copies bites.
            psPair = ctx.enter_context(
                tc.tile_pool(name="psPair", bufs=2, space="PSUM"))
            psY = ctx.enter_context(
                tc.tile_pool(name="psY", bufs=2, space="PSUM"))
            psA = ctx.enter_context(
                tc.tile_pool(name="psA", bufs=2, space="PSUM"))
            dram = ctx.enter_context(
                tc.tile_pool(name="dram", bufs=1, space="DRAM"))

            # ---- startup DMA order: the very first matmul only needs
            # x dc-tile 0 + wq dc 0, so those issue first; the remaining
            # x tiles split across the gpsimd/sync queues so no engine's
            # issue backlog delays the qkv ramp
            xt0_dcs = [x0pool.tile([128, IB], BF16, tag="xt0dc",
                                   name=f"xt0dc{dc}") for dc in range(8)]
            wq_dcs = [wpool.tile([128, 3 * FL], BF16, tag=f"wq{dc}",
                                 name=f"wq{dc}") for dc in range(8)]
            nc.gpsimd.dma_start(xt0_dcs[0][:], xT_r[:, 0, 0:IB])
            nc.sync.dma_start(wq_dcs[0][:], wqkvT_r[:, 0, :])
            nc.scalar.dma_start(xt0_dcs[1][:], xT_r[:, 1, 0:IB])
            nc.gpsimd.dma_start(xt0_dcs[2][:], xT_r[:, 2, 0:IB])
            nc.scalar.dma_start(xt0_dcs[3][:], xT_r[:, 3, 0:IB])
            nc.sync.dma_start(wq_dcs[1][:], wqkvT_r[:, 1, :])
            # tri[p, f] = 1.0 where f >= p else 0 (keep key j0+p for query
            # j0+f); needed by the first diagonal mask (~13us in)
            tri = const.tile([128, 128], BF16, tag="tri")
            nc.gpsimd.memset(tri[:], 1.0)
            nc.gpsimd.affine_select(
                out=tri[:], in_=tri[:],
                compare_op=mybir.AluOpType.is_ge,
                fill=0.0, base=0,
                pattern=[[1, 128]], channel_multiplier=-1,
            )
            nc.gpsimd.dma_start(xt0_dcs[4][:], xT_r[:, 4, 0:IB])
            nc.scalar.dma_start(xt0_dcs[5][:], xT_r[:, 5, 0:IB])
            nc.gpsimd.dma_start(xt0_dcs[6][:], xT_r[:, 6, 0:IB])
            nc.sync.dma_start(wq_dcs[2][:], wqkvT_r[:, 2, :])
            nc.scalar.dma_start(xt0_dcs[7][:], xT_r[:, 7, 0:IB])
            nc.sync.dma_start(wq_dcs[3][:], wqkvT_r[:, 3, :])
            for dc in range(4, 8):
                nc.sync.dma_start(wq_dcs[dc][:], wqkvT_r[:, dc, :])
            xt_pre1 = xpool.tile([128, 8, IB], BF16, tag="xt",
                                 name="xt_pre1")
            nc.gpsimd.dma_start(xt_pre1[:], xT_r[:, :, IB:2 * IB])

            # ---- persistent q/k buffers: [128, 2, T], slot 0 = q, slot
            # 1 = k. Head 0 lives on partitions 0:64, head 1 on 64:128 of
            # both slots -- the packed score matmuls contract only their
            # own head's 64 rows, so no zero padding is needed anywhere.
            qkvTs = [qkvpool.tile([128, 2, T], BF16, tag=f"qkvT{i}",
                                  name=f"qkvT{i}") for i in range(3)]
            # vt2[tok, hl, jt, 0:64] = ones (denominator rows -> psy
            # partitions 0:64, pre-broadcast for the normalize multiply),
            # 64:128 = v. Both heads in one tile so the per-token-tile v
            # copy is a single DVE op. Double-buffered per batch parity.
            vt2_bufs = []
            for i in range(2):
                vt = vpool.tile([128, HL, NJ, 128], BF16, tag=f"vt2{i}",
                                name=f"vt2{i}")
                if i == 0:
                    nc.vector.memset(vt[:, :, :, 0:DH], 1.0)
                else:
                    # buf 1 is first used by batch 1 -- the gpsimd queue
                    # clears its startup DMA backlog well before
                    nc.gpsimd.memset(vt[:, :, :, 0:DH], 1.0)
                vt2_bufs.append(vt)

            wout_sb = wpool.tile([128, 8, D], BF16, tag="wout")

            # ---- internal DRAM for the per-batch staged AllToAll ----
            a2a_ins = [dram.tile([N_CORES, FL, CH], BF16,
                                 tag=f"a2a_in{s}", name=f"a2a_in{s}")
                       for s in range(B)]
            a2a_outs = [dram.tile([N_CORES, FL, CH], BF16,
                                  tag=f"a2a_out{s}", name=f"a2a_out{s}")
                        for s in range(B)]
            # batch 3 splits into two half-batch collectives of 128-token
            # chunks so its first half overlaps the rest of its attention
            a2a3_ins = [dram.tile([N_CORES, FL, CH3], BF16,
                                  tag=f"a2a3_in{h}", name=f"a2a3_in{h}")
                        for h in range(2)]
            a2a3_outs = [dram.tile([N_CORES, FL, CH3], BF16,
                                   tag=f"a2a3_out{h}", name=f"a2a3_out{h}")
                         for h in range(2)]

            # ---- PE queue pinning: the Tile scheduler is priority-driven
            # and would hoist dependency-free qkv/outproj matmuls ahead of
            # the attention stream; ordering-only deps pin the PE queue to
            # the emission order, which is the schedule designed below.
            prev_pe = [None]

            def _mm(*a, **k):
                inst = nc.tensor.matmul(*a, **k)
                if prev_pe[0] is not None:
                    add_dep_helper(inst.ins, prev_pe[0], sync=False,
                                   reason="pe queue order")
                prev_pe[0] = inst.ins
                return inst

            # the normalize+staging chain of attention block X is emitted
            # early in block X+1 (or at batch end), so its psum-reading
            # DVE ops never sit ahead of mask multiplies the PE waits on
            pending_norms = []

            def _flush_norms():
                while pending_norms:
                    pending_norms.pop(0)()

            def _make_norm(b, ib, hl, psy):
                def _norm():
                    # psy rows 0:64 all hold the denominator (ones block),
                    # so the reciprocal lands pre-broadcast for the mult
                    den_rec = ydpool.tile([DH, IB], F32, tag="den_rec",
                                          name=f"den_rec_{b}_{ib}_{hl}")
                    nc.vector.reciprocal_approx_fast(den_rec[:],
                                                     psy[0:DH, :])
                    yd = ydpool.tile([DH, IB], BF16, tag="yd",
                                     name=f"yd_{b}_{ib}_{hl}")
                    nc.vector.tensor_tensor(yd[:], psy[DH:128, :],
                                            den_rec[:],
                                            mybir.AluOpType.mult)
                    r0 = hl * DH
                    # one DMA per norm: the dst rank axis is transposed
                    # into the AP so yd's (row, chunk*tok) order matches.
                    # Staging stays on gpsimd: a dependency-waiting DMA
                    # issue head-of-line-blocks its queue, and sync
                    # carries the latency-critical x prefetches.
                    if b < B - 1:
                        dst = a2a_ins[b][2 * ib:2 * ib + 2, r0:r0 + DH, :]
                        nc.gpsimd.dma_start(
                            dst.rearrange("c p t -> p c t"), yd[:])
                    else:
                        # batch 3: 128-token chunks, half buffers
                        c0_ = (ib % 2) * 4
                        dst = a2a3_ins[ib // 2][c0_:c0_ + 4, r0:r0 + DH, :]
                        nc.gpsimd.dma_start(
                            dst.rearrange("c p t -> p c t"), yd[:])
                return _norm

            # ---- qkv projection for one 512-token block, as 8 thunks
            # (2 half-groups each for q and k, 4 token-tile groups for v)
            # that interleave into the previous block's attention slots
            def _qkv_chunks(b, tb, qkvT, vt2):
                st = {}
                sl = tb * IB

                def x_ap(dc, t0=0, t1=IB):
                    if b == 0 and tb == 0:
                        return xt0_dcs[dc][:, t0:t1]
                    if b == 0 and tb == 1:
                        return xt_pre1[:, dc, t0:t1]
                    return st["xt"][:, dc, t0:t1]

                def proj_half(ft, lo, hi):
                    def f():
                        if ft == 0 and lo == 0:
                            if not (b == 0 and tb <= 1):
                                xt = xpool.tile([128, 8, IB], BF16,
                                                tag="xt",
                                                name=f"xt_{b}_{tb}")
                                t0 = b * T + tb * IB
                                nc.sync.dma_start(xt[:],
                                                  xT_r[:, :, t0:t0 + IB])
                                st["xt"] = xt
                            if b == 0 and tb == 3:
                                # big resident load deferred off the
                                # DMA-bandwidth-starved startup ramp;
                                # outproj only needs it ~150us later
                                nc.gpsimd.dma_start(wout_sb[:], woutT_r)
                        key = "psq" if ft == 0 else "psk"
                        if lo == 0:
                            st[key] = psA.tile([128, IB], F32, tag="ps",
                                               name=f"ps_{b}_{tb}_{ft}")
                        ps = st[key]
                        for dc in range(lo, hi):
                            _mm(ps[:],
                                lhsT=wq_dcs[dc][:, ft * 128:(ft + 1) * 128],
                                rhs=x_ap(dc),
                                start=(dc == 0), stop=(dc == 7))
                        if hi == 8:
                            nc.vector.tensor_copy(
                                qkvT[:, ft, sl:sl + IB], ps[:])
                    return f

                def v_mms(q4):
                    def f():
                        # direct [token, feature] projection: x tokens as
                        # the stationary, w_v columns stream -- v arrives
                        # already transposed for the PV stationary
                        if q4 == 0:
                            st["psv"] = psA.tile([128, IB], F32, tag="ps",
                                                 name=f"psv_{b}_{tb}")
                        seg = st["psv"][:, q4 * 128:(q4 + 1) * 128]
                        for dc in range(8):
                            _mm(seg,
                                lhsT=x_ap(dc, q4 * 128, (q4 + 1) * 128),
                                rhs=wq_dcs[dc][:, 256:384],
                                start=(dc == 0), stop=(dc == 7))
                    return f

                def v_copies():
                    # all copies after all v matmul groups: the psum WAR
                    # tracking is bank-granular, so interleaving copies
                    # between the quarter groups would serialize the PE
                    # behind the DVE once per quarter. One copy per token
                    # tile covers both heads (seg cols = [v_h0 | v_h1]).
                    for q4 in range(4):
                        seg = st["psv"][:, q4 * 128:(q4 + 1) * 128]
                        jt = tb * 4 + q4
                        nc.vector.tensor_copy(
                            vt2[:, :, jt, DH:128],
                            seg.rearrange("p (c t) -> p c t", c=2))

                return [proj_half(0, 0, 4), proj_half(0, 4, 8),
                        proj_half(1, 0, 4), proj_half(1, 4, 8),
                        v_mms(0), v_mms(1), v_mms(2), v_mms(3),
                        v_copies]

            # ---- output projection, as per-(tt, db) chunks ----
            op_lh = {}

            def _op_chunk(s, tt, db, buf, chs, row_base, pool):
                def f():
                    key = (s, row_base)
                    if key not in op_lh:
                        lh = opool.tile([128, 8, CH], BF16, tag="lh",
                                        name=f"lh_{s}_{row_base}")
                        lh = lh[:, :, :chs]
                        nc.sync.dma_start(
                            lh[:], buf[:].rearrange("p f t -> f p t"))
                        op_lh[key] = lh
                    lh = op_lh[key]
                    nrow = min(128, chs)
                    pst = pool.tile(
                        [128, 2, IB] if pool is psPair else [128, IB],
                        F32, tag="pair" if pool is psPair else "ps",
                        name=f"pso_{s}_{row_base}_{tt}_{db}")
                    pso = pst[:, 0, :] if pool is psPair else pst
                    for fc in range(8):
                        _mm(pso[:nrow, :],
                            lhsT=lh[:, fc, tt * nrow:(tt + 1) * nrow],
                            rhs=wout_sb[:, fc, db * IB:(db + 1) * IB],
                            start=(fc == 0), stop=(fc == 7))
                    osb = opool.tile([128, IB], BF16, tag="osb",
                                     name=f"osb_{s}_{row_base}_{tt}_{db}")
                    nc.vector.tensor_copy(osb[:nrow, :], pso[:nrow, :])
                    row0 = row_base + tt * nrow
                    nc.sync.dma_start(
                        out[row0:row0 + nrow, db * IB:(db + 1) * IB],
                        osb[:nrow, :])
                return f

            def _op_chunks(s, buf=None, chs=CH, row_base=None, pool=None):
                if buf is None:
                    buf = a2a_outs[s]
                if row_base is None:
                    row_base = s * CH
                nrow = min(128, chs)
                return [_op_chunk(s, tt, db, buf, chs, row_base, pool)
                        for tt in range(chs // nrow)
                        for db in range(D // IB)]

            # ---- causal attention for one (batch, 512-token block):
            # packed score pairs + paired exp + interleaved PV, with the
            # next block's qkv chunks spread between the slots
            def _attn_block(b, ib, qkvT, vt2, chunks):
                nj = 4 * (ib + 1)
                nchunk = len(chunks)
                emitted = 0
                psys = []
                p_tiles = {}

                def pv_pair(jt):
                    c0 = max(0, (jt - ib * 4) * 128)
                    p = p_tiles.pop(jt)
                    for hl in range(HL):
                        _mm(psys[hl][:, c0:], lhsT=vt2[:, hl, jt, :],
                            rhs=p[:, hl, c0:],
                            start=(jt == 0), stop=(jt == nj - 1))

                for jt in range(nj):
                    c0 = max(0, (jt - ib * 4) * 128)
                    pair = psPair.tile([128, 2, IB], F32, tag="pair",
                                       name=f"pair_{b}_{ib}_{jt}")
                    for hl in range(HL):
                        lo, hi = hl * 64, (hl + 1) * 64
                        _mm(pair[:, hl, c0:],
                            lhsT=qkvT[lo:hi, 1, jt * 128:(jt + 1) * 128],
                            rhs=qkvT[lo:hi, 0, ib * IB + c0:(ib + 1) * IB],
                            start=True, stop=True)
                    p = ppool.tile([128, 2, IB], BF16, tag="p",
                                   name=f"p_{b}_{ib}_{jt}")
                    nc.scalar.activation(
                        p[:, :, c0:], pair[:, :, c0:],
                        mybir.ActivationFunctionType.Exp, scale=SCALE)
                    if jt >= ib * 4:  # diagonal: triangular mask
                        for hl in range(HL):
                            nc.vector.tensor_tensor(
                                p[:, hl, c0:c0 + 128],
                                p[:, hl, c0:c0 + 128],
                                tri[:], mybir.AluOpType.mult)
                    p_tiles[jt] = p
                    if jt == 0:
                        for hl in range(HL):
                            psys.append(psY.tile(
                                [128, IB], F32, tag="psy",
                                name=f"psy_{b}_{ib}_{hl}"))
                    if jt == 1:
                        _flush_norms()
                    want = (jt + 1) * nchunk // nj
                    while emitted < want:
                        chunks[emitted]()
                        emitted += 1
                    if jt >= PV_LAG:
                        pv_pair(jt - PV_LAG)
                for jt in range(max(0, nj - PV_LAG), nj):
                    pv_pair(jt)
                for hl in range(HL):
                    pending_norms.append(_make_norm(b, ib, hl, psys[hl]))

            # ---- main emission ----
            op_stage_chunks = (_op_chunks(0, pool=psPair)
                               + _op_chunks(1, pool=psPair)
                               + _op_chunks(2, pool=psPair)
                               + _op_chunks(B - 1, buf=a2a3_outs[0],
                                            chs=CH3, row_base=(B - 1) * CH,
                                            pool=psPair)
                               + _op_chunks(B - 1, buf=a2a3_outs[1],
                                            chs=CH3,
                                            row_base=(B - 1) * CH + CH3,
                                            pool=psPair))

            for f in _qkv_chunks(0, 0, qkvTs[0], vt2_bufs[0]):
                f()
            for b in range(B):
                qkvT = qkvTs[b % 3]
                vt2 = vt2_bufs[b % 2]
                for tb in range(T // IB):
                    if tb < 3:
                        nxt = _qkv_chunks(b, tb + 1, qkvT, vt2)
                    elif b < B - 1:
                        nq = qkvTs[(b + 1) % 3]
                        nxt = _qkv_chunks(b + 1, 0, nq,
                                          vt2_bufs[(b + 1) % 2])
                    else:
                        # last block: keep all outproj for the tail so the
                        # PE has maximal ungated work to chew while the
                        # final half-collective is in flight
                        nxt = []
                    _attn_block(b, tb, qkvT, vt2, nxt)
                    if b == B - 1 and tb % 2 == 1:
                        _flush_norms()
                        nc.gpsimd.collective_compute(
                            "AllToAll", mybir.AluOpType.bypass,
                            replica_groups=[list(range(N_CORES))],
                            ins=[a2a3_ins[tb // 2][:]],
                            outs=[a2a3_outs[tb // 2][:]],
                        )
                if b < B - 1:
                    # stage collective fires as soon as batch b's y landed
                    _flush_norms()
                    nc.gpsimd.collective_compute(
                        "AllToAll", mybir.AluOpType.bypass,
                        replica_groups=[list(range(N_CORES))],
                        ins=[a2a_ins[b][:]], outs=[a2a_outs[b][:]],
                    )
            # outproj chunks fill the final-collective wait
            for f in op_stage_chunks:
                f()

    nc.finalize()
    return nc


_NC_CACHE = {}


def _get_nc():
    if "nc" not in _NC_CACHE:
        _NC_CACHE["nc"] = _build()
    return _NC_CACHE["nc"]


def kernel(x, w_qkv, w_out):
    x = np.asarray(x, np.float32).reshape(BT, D)
    w_qkv = np.asarray(w_qkv, np.float32)
    w_out = np.asarray(w_out, np.float32)

    xT = np.ascontiguousarray(x.T).astype(ml_dtypes.bfloat16)
    woutT = np.ascontiguousarray(w_out.T).astype(ml_dtypes.bfloat16)

    in_maps = []
    for c in range(N_CORES):
        rows = []
        for t in range(3):
            for hl in range(HL):
                h = HL * c + hl
                rows.append(w_qkv[t * H * DH + h * DH:
                                  t * H * DH + (h + 1) * DH])
        wq_c = np.concatenate(rows, axis=0)  # [384, D]
        in_maps.append({
            "xT": xT,
            "wqkvT": np.ascontiguousarray(wq_c.T).astype(ml_dtypes.bfloat16),
            "woutT": woutT,
        })

    nc = _get_nc()
    do_trace = bool(os.environ.get("ATTN_TRACE"))
    if do_trace:
        _install_profile_hook()
    res = run_bass_kernel_spmd(nc, in_maps, list(range(N_CORES)),
                               trace=do_trace)
    if res.exec_time_ns is not None:
        print(f"HW exec time: {res.exec_time_ns} ns")
        _NC_CACHE["exec_time_ns"] = res.exec_time_ns
        _NC_CACHE["trace"] = res.instructions_and_trace
    # rank r's out rows: batches 0-2 are 256-token chunks (token
    # b*T + r*256 + t); batch 3 is two 128-token chunks, one per
    # half-batch (tokens 3*T + h*1024 + r*128 + t)
    full = np.empty((BT, D), np.float32)
    for c in range(N_CORES):
        o = np.asarray(res.results[c]["out"], np.float32)
        for b in range(B - 1):
            full[b * T + c * CH:(b * T) + (c + 1) * CH] = \
                o[b * CH:(b + 1) * CH]
        b3 = (B - 1) * CH
        for h in range(2):
            dst = (B - 1) * T + h * (T // 2) + c * CH3
            full[dst:dst + CH3] = o[b3 + h * CH3:b3 + (h + 1) * CH3]
    return full.reshape(B, T, D)
